# revision 1
# baseline (speedup 1.0000x reference)
"""Distributed softmax-attention readout (NeuralDictionary) on 8 trn2 cores.

Math: out = softmax(-sum|keys - q|) @ values over N=200000 rows, D=128.

Strategy:
  - Shard rows across 8 cores (25000 rows/core, padded to 25088 = 7*128*28).
  - Per core (Bass/Tile kernel):
      pass 1: stream keys, scores t_j = -sum_d |k_jd - q_d|   (DVE sub + abs-sum-reduce)
      max:    per-partition max -> PE transpose -> global max -> PE broadcast
      exp:    e = exp(t - m) with accumulated partial sums     (ACT, one op)
      pass 2: stream values, out_partial = e @ V               (PE matmuls into PSUM)
    Outputs per core: outvec [1,128] = sum_j e_j * V_j, stats [128,2] = (z_p, m).
  - Host combines the 8 (vec, Z, m) triples exactly (tiny, numpy float64).

Layout: rows are blocked as row = b*(128*28) + p*28 + r so every DMA is
128 partitions x 14 KiB contiguous (1.75 MiB per dma_start, near line rate).
"""

import sys

import numpy as np

try:
    from concourse import bacc, bass, mybir, tile
    from concourse import bass_utils
except ImportError:  # pragma: no cover
    sys.path.insert(0, "/opt/trn_rl_repo")
    from concourse import bacc, bass, mybir, tile
    from concourse import bass_utils

F32 = mybir.dt.float32
P = 128          # partitions
D = 128          # feature dim
NCORES = 8
N_TOTAL = 200000
PER_CORE = N_TOTAL // NCORES          # 25000
RPP = 28                              # rows per partition per block
NBLK = 7                              # blocks
NPAD = P * RPP * NBLK                 # 25088 padded rows per core
COLS = RPP * NBLK                     # 196 score columns per partition
PAD_KEY = 100.0                       # padded key value -> huge L1 -> weight 0

_CACHE: dict = {}


def build_nc():
    nc = bacc.Bacc("TRN2", target_bir_lowering=False, debug=False)

    kd = nc.dram_tensor("keys", (NPAD, D), F32, kind="ExternalInput")
    vd = nc.dram_tensor("values", (NPAD, D), F32, kind="ExternalInput")
    qd = nc.dram_tensor("qrep", (P, D), F32, kind="ExternalInput")
    idd = nc.dram_tensor("ident", (P, P), F32, kind="ExternalInput")
    ond = nc.dram_tensor("ones", (1, P), F32, kind="ExternalInput")
    ovd = nc.dram_tensor("outvec", (1, D), F32, kind="ExternalOutput")
    osd = nc.dram_tensor("stats", (P, 2), F32, kind="ExternalOutput")

    AX = mybir.AxisListType
    OP = mybir.AluOpType
    ACT = mybir.ActivationFunctionType

    with tile.TileContext(nc) as tc:
        with (
            tc.tile_pool(name="const", bufs=1) as const,
            tc.tile_pool(name="kp", bufs=3) as kpool,
            tc.tile_pool(name="vp", bufs=NBLK) as vpool,
            tc.tile_pool(name="sp", bufs=1) as spool,
            tc.tile_pool(name="ps", bufs=1, space="PSUM") as psum,
        ):
            qrep = const.tile([P, D], F32, tag="qrep")
            nc.sync.dma_start(qrep[:], qd.ap())
            ident = const.tile([P, P], F32, tag="ident")
            nc.sync.dma_start(ident[:], idd.ap())
            ones = const.tile([1, P], F32, tag="ones")
            nc.sync.dma_start(ones[:], ond.ap())

            kview = kd.ap().rearrange("(b p r) d -> b p r d", b=NBLK, p=P)
            vview = vd.ap().rearrange("(b p r) d -> b p r d", b=NBLK, p=P)

            scores = spool.tile([P, COLS], F32, tag="scores")
            qb = qrep[:].unsqueeze(1).broadcast_to((P, RPP, D))

            # pass 1: keys -> scores
            for b in range(NBLK):
                kt = kpool.tile([P, RPP, D], F32, tag="kt")
                nc.sync.dma_start(kt[:], kview[b])
                nc.vector.tensor_tensor(kt[:], kt[:], qb, OP.subtract)
                nc.vector.tensor_reduce(
                    scores[:, b * RPP:(b + 1) * RPP],
                    kt[:],
                    axis=AX.X,
                    op=OP.add,
                    apply_absolute_value=True,
                    negate=True,
                )

            # stream values in (consumed by the matmuls at the end)
            vts = []
            for b in range(NBLK):
                vt = vpool.tile([P, RPP, D], F32, tag="vt")
                nc.sync.dma_start(vt[:], vview[b])
                vts.append(vt)

            # global max of scores (cross-partition via PE)
            mp = spool.tile([P, 1], F32, tag="mp")
            nc.vector.tensor_reduce(mp[:], scores[:], axis=AX.X, op=OP.max)
            pt = psum.tile([1, P], F32, tag="pt")
            nc.tensor.matmul(pt[:], mp[:], ident[:], start=True, stop=True)
            m1 = spool.tile([1, 1], F32, tag="m1")
            nc.vector.tensor_reduce(m1[:], pt[:], axis=AX.X, op=OP.max)
            pb = psum.tile([P, 1], F32, tag="pb")
            nc.tensor.matmul(pb[:], ones[:], m1[:], start=True, stop=True)

            negm = spool.tile([P, 1], F32, tag="negm")
            nc.vector.tensor_scalar_mul(negm[:], pb[:], -1.0)
            clamp = spool.tile([P, 1], F32, tag="clamp")
            nc.vector.tensor_scalar_add(clamp[:], pb[:], -80.0)

            # clamp scores (keeps padded rows inside the ACT exp LUT range)
            nc.vector.tensor_scalar_max(scores[:], scores[:], clamp[:])

            # e = exp(t - m), z_p = per-partition sum of e
            stats_sb = spool.tile([P, 2], F32, tag="stats")
            e = spool.tile([P, COLS], F32, tag="e")
            nc.scalar.activation(
                e[:], scores[:], ACT.Exp,
                bias=negm[:], scale=1.0,
                accum_out=stats_sb[:, 0:1],
            )
            nc.vector.tensor_copy(stats_sb[:, 1:2], pb[:])

            # pass 2: out = e @ V  (contract over rows = partitions, per column)
            pvec = psum.tile([1, D], F32, tag="pvec")
            for b in range(NBLK):
                for r in range(RPP):
                    col = b * RPP + r
                    nc.tensor.matmul(
                        pvec[:],
                        e[:, col:col + 1],
                        vts[b][:, r, :],
                        start=(col == 0),
                        stop=(col == COLS - 1),
                    )

            svec = spool.tile([1, D], F32, tag="svec")
            nc.vector.tensor_copy(svec[:], pvec[:])
            nc.sync.dma_start(ovd.ap(), svec[:])
            nc.sync.dma_start(osd.ap(), stats_sb[:])

    nc.compile()
    return nc


def get_nc():
    if "nc" not in _CACHE:
        _CACHE["nc"] = build_nc()
    return _CACHE["nc"]


def make_in_maps(query, keys, values):
    query = np.ascontiguousarray(np.asarray(query, dtype=np.float32))
    keys = np.ascontiguousarray(np.asarray(keys, dtype=np.float32))
    values = np.ascontiguousarray(np.asarray(values, dtype=np.float32))

    qrep = np.tile(query[None, :], (P, 1))
    ident = np.eye(P, dtype=np.float32)
    ones = np.ones((1, P), dtype=np.float32)

    in_maps = []
    for c in range(NCORES):
        ks = keys[c * PER_CORE:(c + 1) * PER_CORE]
        vs = values[c * PER_CORE:(c + 1) * PER_CORE]
        kp = np.full((NPAD, D), PAD_KEY, dtype=np.float32)
        kp[:PER_CORE] = ks
        vp = np.zeros((NPAD, D), dtype=np.float32)
        vp[:PER_CORE] = vs
        in_maps.append({
            "keys": kp, "values": vp,
            "qrep": qrep, "ident": ident, "ones": ones,
        })
    return in_maps


def combine(results):
    """results: list of 8 dicts with 'outvec' [1,128] and 'stats' [128,2]."""
    M = np.array([r["stats"][0, 1] for r in results], dtype=np.float64)
    Z = np.array(
        [r["stats"][:, 0].astype(np.float64).sum() for r in results],
        dtype=np.float64,
    )
    V = np.stack([r["outvec"][0].astype(np.float64) for r in results])
    Mg = M.max()
    w = np.exp(M - Mg)
    out = (w[:, None] * V).sum(axis=0) / (w * Z).sum()
    return out.astype(np.float32)


def kernel(query, keys, values):
    in_maps = make_in_maps(query, keys, values)
    res = bass_utils.run_bass_kernel_spmd(
        get_nc(), in_maps, core_ids=list(range(NCORES))
    )
    return combine(res.results)


if __name__ == "__main__":
    rng = np.random.default_rng(0)
    q = rng.standard_normal(D).astype(np.float32)
    k = rng.standard_normal((N_TOTAL, D)).astype(np.float32)
    v = rng.standard_normal((N_TOTAL, D)).astype(np.float32)
    out = kernel(q, k, v)
    print(out[:8])
